# revision 21
# baseline (speedup 1.0000x reference)
"""LoRA attention kernel for 8 Trainium2 NeuronCores.

Sharding: data-parallel over batch B=2 (cores 0-3 -> b=0, cores 4-7 -> b=1),
tensor-parallel over heads within each batch group (4 heads/core). LoRA paths
and q/v base linears are folded host-side into effective projection weights.

Optimizations over the fp32r baseline (668us -> this version):
- All matmul operands bf16 (fp32 PSUM accumulation); tolerance is 2e-2 and
  measured error is ~4e-3, so bf16 is safe and loads/streams/DMAs halve.
- Key/value tokens are compacted host-side using the padding mask (~half of
  the 2048 keys are masked), halving QK/exp/PV work exactly (padded tail
  keys carry mask 0 and contribute zero to both numerator and denominator).
- Scores are computed transposed (ST[m, n]) so the softmax denominator and
  key mask fold into the P@V matmul via an augmented v column.
- Q is kept in a zero-padded [128]-partition layout (each head's 64 dims in
  its parity half, zeros in the other) so QK contracts over 128 partitions:
  K=64 matmuls stream at half rate on the PE, K=128 at full rate.
- One flat software pipeline: the attention stream (QK -> exp -> lagged PV)
  is issued tile-by-tile with the remaining projection work (K/V/Q row tiles
  not needed yet) interleaved in small pieces, so the ACT engine starts
  exp'ing ~40us earlier and the in-order PE queue never blocks on a
  not-ready instruction. PSUM: 2x st[128,1024] + 2x op[65,1024] = 8 banks;
  projection chunks share the st pool's buffers.
- V transposes run on the DMA engine (XBAR dma_start_transpose), not the PE.
- The per-(head,half) drain copies the unnormalized output and denominator
  out of PSUM immediately (freeing the op buffer), then the approximate
  reciprocal (exact one is 6 cyc/elem and stalled the PE 13us/head) and the
  stride-0 DRAM broadcast produce the normalized bf16 attention output.
- Per-head bf16 AllGathers overlap the collectives with later heads'
  compute; gathered-head loads are issued after all attention DMAs so they
  never block the reciprocal round-trips in the FIFO DMA queue.
- The output projection contracts the gathered heads (w_out rows permuted
  host-side to match gather order) right after the last AllGather.
"""

import sys
from contextlib import ExitStack

import numpy as np

for _p in ("/opt/trn_rl_repo", "/opt/trn_rl_repo/concourse"):
    if _p not in sys.path:
        sys.path.insert(0, _p)

import concourse.bass as bass
import concourse.mybir as mybir
import concourse.tile as tile
from concourse import bacc
from concourse import bass_utils

import ml_dtypes

F32 = mybir.dt.float32
BF16 = mybir.dt.bfloat16
EXP = mybir.ActivationFunctionType.Exp
BFNP = ml_dtypes.bfloat16

H, D, DIM, R = 16, 64, 1024, 10
B, N = 2, 2048
NCORES = 8
GROUPS = [[0, 1, 2, 3], [4, 5, 6, 7]]
HPC = H // 4          # heads per core
HD = HPC * D          # 256 projection rows per core
ATT = float(D) ** -0.5
LS = 1.0 / R

KT = DIM // 128       # 8 contraction tiles
NCH = N // 512        # 4 query chunks of 512
# AllGather head order: agout[h] rows are rank-major, so the out-projection
# contraction sees global heads in this order (w_out rows permuted to match).
AG_HEAD_ORDER = [r * HPC + h for h in range(HPC) for r in range(4)]

# test harness hooks
TRACE = False
TRACE_DIR = None
LAST_RESULTS = None

_NC_CACHE = {}


def _chunks(total, step):
    return [(c0, min(c0 + step, total)) for c0 in range(0, total, step)]


def _build_nc(M, debug=False):
    MT = M // 128
    MCH = _chunks(M, 512)
    SEGS = [(h, half) for h in range(HPC) for half in range(2)]
    dbg = "ExternalOutput" if debug else "Internal"

    nc = bacc.Bacc(None, target_bir_lowering=False, num_devices=NCORES)

    xT = nc.dram_tensor("xT", (DIM, N), BF16, kind="ExternalInput")
    xkT = nc.dram_tensor("xkT", (DIM, M), BF16, kind="ExternalInput")
    wqT = nc.dram_tensor("wqT", (DIM, HD), BF16, kind="ExternalInput")
    wkvT = nc.dram_tensor("wkvT", (DIM, 2 * HD), BF16, kind="ExternalInput")
    pbq = nc.dram_tensor("pbq", (HD,), F32, kind="ExternalInput")
    pbv = nc.dram_tensor("pbv", (HD,), F32, kind="ExternalInput")
    mk = nc.dram_tensor("mk", (M,), F32, kind="ExternalInput")
    woT = nc.dram_tensor("woT", (DIM, HD), BF16, kind="ExternalInput")
    bo = nc.dram_tensor("bo", (HD,), F32, kind="ExternalInput")
    outT = nc.dram_tensor("outT", (HD, N), F32, kind="ExternalOutput")

    agin = nc.dram_tensor("agin", (HPC * D, N), BF16)
    agout = nc.dram_tensor("agout", (HPC, 4 * D, N), BF16)
    recd = nc.dram_tensor("recd", (HPC, N), F32, kind=dbg)

    with ExitStack() as ctx:
        tc = ctx.enter_context(tile.TileContext(nc))
        const = ctx.enter_context(tc.tile_pool(name="const", bufs=1))

        pbq_sb = const.tile([128, 2], F32)
        nc.sync.dma_start(out=pbq_sb, in_=pbq[:].rearrange("(i p) -> p i", p=128))
        pbv_sb = const.tile([128, 2], F32)
        nc.sync.dma_start(out=pbv_sb, in_=pbv[:].rearrange("(i p) -> p i", p=128))
        mk_sb = const.tile([128, MT], F32)
        nc.sync.dma_start(out=mk_sb, in_=mk[:].rearrange("(t p) -> p t", p=128))
        bo_sb = const.tile([128, 2], F32)
        nc.sync.dma_start(out=bo_sb, in_=bo[:].rearrange("(c p) -> p c", p=128))
        woT_sb = const.tile([128, KT, HD], BF16)
        woT_r = woT[:, :].rearrange("(k p) c -> p k c", p=128)
        for k in range(KT):
            nc.sync.dma_start(out=woT_sb[:, k, :], in_=woT_r[:, k, :])

        # q in zero-padded per-head layout (see module docstring)
        qz = const.tile([128, HPC, N], BF16)
        nc.vector.memset(qz, 0)
        kvT = const.tile([128, 4, M], BF16)      # [k0 k1 v0 v1] row tiles
        vsb = const.tile([128, MT, HPC, D + 1], BF16)  # v.T + mask column
        agT = const.tile([128, KT, N], BF16)     # gathered heads for out proj

        wq_sb = const.tile([128, KT, HD], BF16)
        wkv_sb = const.tile([128, KT, 2 * HD], BF16)
        xk_sb = const.tile([128, KT, M], BF16)
        xT_sb = const.tile([128, KT, N], BF16)
        wq_r = wqT[:, :].rearrange("(k p) m -> p k m", p=128)
        wkv_r = wkvT[:, :].rearrange("(k p) m -> p k m", p=128)
        xk_r = xkT[:, :].rearrange("(k p) n -> p k n", p=128)
        xT_r = xT[:, :].rearrange("(k p) n -> p k n", p=128)
        # Coarse DMA loads (few big transfers: descriptor issue on the sync
        # engine costs ~0.6us per dma_start, so per-chunk splits are a loss),
        # ordered by consumption: K/V weights + compacted kv tokens first.
        nc.sync.dma_start(out=wkv_sb, in_=wkv_r)
        for c0, c1 in MCH:
            nc.sync.dma_start(out=xk_sb[:, :, c0:c1], in_=xk_r[:, :, c0:c1])
        nc.sync.dma_start(out=wq_sb, in_=wq_r)
        for khalf in range(2):
            ksl = slice(khalf * (KT // 2), (khalf + 1) * (KT // 2))
            nc.sync.dma_start(out=xT_sb[:, ksl, :], in_=xT_r[:, ksl, :])

        # ---- phase 1: K/V projections + Q row-tile 0 ----
        # Only K/Q row tile 0 (heads 0-1) must precede the attention stream;
        # row tile 1 (heads 2-3) is interleaved into the stream later.
        with tc.tile_pool(name="vrawp", bufs=4) as vrawp, \
             tc.tile_pool(name="pp_proj", bufs=4, space="PSUM") as ppp:
            pstiles = {}

            def k_chunk(i, ci, pool, klo=0, khi=KT):
                c0, c1 = MCH[ci]
                w = c1 - c0
                key = ("k", i, ci)
                if key not in pstiles:
                    pstiles[key] = pool.tile(
                        [128, 512], F32, tag="ps" if pool is ppp else "st",
                        name=f"psk{i}_{ci}")
                ps = pstiles[key]
                for k in range(klo, khi):
                    nc.tensor.matmul(
                        ps[:, 0:w],
                        lhsT=wkv_sb[:, k, i * 128:(i + 1) * 128],
                        rhs=xk_sb[:, k, c0:c1],
                        start=(k == 0), stop=(k == KT - 1),
                    )
                if khi == KT:
                    nc.vector.tensor_copy(kvT[:, i, c0:c1], ps[:, 0:w])

            def q_chunk(i, nch, pool, klo=0, khi=KT):
                sl = slice(nch * 512, (nch + 1) * 512)
                key = ("q", i, nch)
                if key not in pstiles:
                    pstiles[key] = pool.tile(
                        [128, 512], F32, tag="ps" if pool is ppp else "st",
                        name=f"psq{i}_{nch}")
                ps = pstiles[key]
                for k in range(klo, khi):
                    nc.tensor.matmul(
                        ps,
                        lhsT=wq_sb[:, k, i * 128:(i + 1) * 128],
                        rhs=xT_sb[:, k, sl],
                        start=(k == 0), stop=(k == KT - 1),
                    )
                if khi == KT:
                    nc.vector.tensor_scalar_add(
                        qz[0:64, 2 * i, sl], ps[0:64, :], pbq_sb[0:64, i:i + 1])
                    nc.vector.tensor_scalar_add(
                        qz[64:128, 2 * i + 1, sl], ps[64:128, :],
                        pbq_sb[64:128, i:i + 1])

            for ci in range(len(MCH)):
                k_chunk(0, ci, ppp)
            # V projection (+bias)
            for i in range(2):
                for ci, (c0, c1) in enumerate(MCH):
                    w = c1 - c0
                    ps = ppp.tile([128, 512], F32, tag="ps", name=f"psv{i}_{ci}")
                    for k in range(KT):
                        nc.tensor.matmul(
                            ps[:, 0:w],
                            lhsT=wkv_sb[:, k, HD + i * 128:HD + (i + 1) * 128],
                            rhs=xk_sb[:, k, c0:c1],
                            start=(k == 0), stop=(k == KT - 1),
                        )
                    nc.vector.tensor_scalar_add(
                        kvT[:, 2 + i, c0:c1], ps[:, 0:w], pbv_sb[:, i:i + 1])
            for nch in range(NCH):
                q_chunk(0, nch, ppp)
            # V transpose on the DMA engine (XBAR), j-major so the j=0 pass
            # only waits on V row-tile 0; mask-scale rows and set the
            # augmented mask column.
            for j in range(2):
                for t in range(MT):
                    vr = vrawp.tile([128, 128], BF16, tag="vraw",
                                    name=f"vr{t}_{j}")
                    nc.sync.dma_start_transpose(
                        out=vr, in_=kvT[:, 2 + j, t * 128:(t + 1) * 128])
                    for hh in range(2):
                        h = j * 2 + hh
                        nc.vector.tensor_scalar_mul(
                            vsb[:, t, h, 0:D],
                            vr[:, hh * D:(hh + 1) * D],
                            mk_sb[:, t:t + 1],
                        )
                    if j == 1:
                        for h in range(HPC):
                            nc.vector.tensor_copy(vsb[:, t, h, D:D + 1],
                                                  mk_sb[:, t:t + 1])

        # ---- phase 2: attention stream (flat pipeline, PV lags QK by 2) ----
        with tc.tile_pool(name="expool", bufs=6) as expool, \
             tc.tile_pool(name="attup", bufs=2) as attup, \
             tc.tile_pool(name="attp", bufs=2) as attp, \
             tc.tile_pool(name="recbp", bufs=2) as recbp, \
             tc.tile_pool(name="recp", bufs=1) as recp, \
             tc.tile_pool(name="pp_o", bufs=2, space="PSUM") as ppo, \
             tc.tile_pool(name="pp_st", bufs=2, space="PSUM") as ppst:

            def drain(op, h, half, nsl):
                # Copy the unnormalized output + denominator out of PSUM
                # right away (frees the op buffer), then normalize via the
                # approximate reciprocal and a stride-0 DRAM broadcast.
                att_u = attup.tile([D, 1024], BF16, tag="attu",
                                   name=f"attu{h}_{half}")
                nc.vector.tensor_copy(att_u, op[0:D, :])
                den = recp.tile([1, 1024], F32, tag="den",
                                name=f"den{h}_{half}")
                nc.vector.tensor_copy(den, op[D:D + 1, :])
                rec = recp.tile([1, 1024], F32, tag="rec",
                                name=f"rec{h}_{half}")
                nc.vector.reciprocal_approx_fast(rec, den)
                nc.sync.dma_start(out=recd[h:h + 1, nsl], in_=rec)
                recb = recbp.tile([D, 1024], F32, tag="recb",
                                  name=f"recb{h}_{half}")
                rsrc = recd[h:h + 1, nsl]
                nc.sync.dma_start(
                    out=recb,
                    in_=bass.AP(tensor=rsrc.tensor, offset=rsrc.offset,
                                ap=[[0, D], [1, 1024]]),
                )
                att = attp.tile([D, 1024], BF16, tag="att",
                                name=f"att{h}_{half}")
                nc.vector.tensor_mul(att, att_u, recb)
                nc.sync.dma_start(out=agin[h * D:(h + 1) * D, nsl], in_=att)
                if half == 1:
                    nc.gpsimd.collective_compute(
                        "AllGather", mybir.AluOpType.bypass,
                        replica_groups=GROUPS,
                        ins=[agin[h * D:(h + 1) * D, :].opt()],
                        outs=[agout[h, :, :].opt()],
                    )

            pend = []

            def make_pv(op, h, half, t, ex, nsl):
                def f():
                    for c in range(2):
                        nc.tensor.matmul(
                            op[:, c * 512:(c + 1) * 512],
                            lhsT=vsb[:, t, h, :],
                            rhs=ex[:, c * 512:(c + 1) * 512],
                            start=(t == 0), stop=(t == MT - 1),
                        )
                    if t == MT - 1:
                        drain(op, h, half, nsl)
                return f

            projq = ([("k", 1, ci, 0, KT) for ci in range(len(MCH))]
                     + [("q", 1, nch, 0, KT) for nch in range(NCH)])

            for si, (h, half) in enumerate(SEGS):
                nsl = slice(half * 1024, (half + 1) * 1024)
                op = ppo.tile([D + 1, 1024], F32, tag="op",
                              name=f"op{h}_{half}")
                for t in range(MT):
                    st = ppst.tile([128, 1024], F32, tag="st",
                                   name=f"st{h}_{half}_{t}")
                    for c in range(2):
                        nc.tensor.matmul(
                            st[:, c * 512:(c + 1) * 512],
                            lhsT=kvT[:, h // 2, t * 128:(t + 1) * 128],
                            rhs=qz[:, h, half * 1024 + c * 512:
                                   half * 1024 + (c + 1) * 512],
                            start=True, stop=True,
                        )
                    ex = expool.tile([128, 1024], BF16, tag="ex",
                                     name=f"ex{h}_{half}_{t}")
                    nc.scalar.activation(ex, st, EXP)
                    if si >= 2 and projq:
                        p = projq.pop(0)
                        if p[0] == "k":
                            k_chunk(p[1], p[2], ppst, p[3], p[4])
                        else:
                            q_chunk(p[1], p[2], ppst, p[3], p[4])
                    pend.append(make_pv(op, h, half, t, ex, nsl))
                    if len(pend) > 2:
                        pend.pop(0)()
            while pend:
                pend.pop(0)()

        # ---- output projection slice ----
        with tc.tile_pool(name="outp", bufs=2) as outp, \
             tc.tile_pool(name="pp_f", bufs=2, space="PSUM") as ppf:
            # gathered-head loads issued after all attention DMAs: the DMA
            # queue is FIFO, so issuing them earlier would block the
            # reciprocal round-trips behind collective waits.
            for h in range(HPC):
                ag_r = agout[h, :, :].rearrange("(kk p) n -> p kk n", p=128)
                for kk in range(2):
                    nc.sync.dma_start(out=agT[:, 2 * h + kk, :],
                                      in_=ag_r[:, kk, :])
            out_r = outT[:, :].rearrange("(c p) n -> p c n", p=128)
            for c in range(2):
                fp = ppf.tile([128, N], F32, tag="fp", name=f"fp{c}")
                for k in range(KT):
                    lhs = woT_sb[:, k, c * 128:(c + 1) * 128]
                    for nch in range(NCH):
                        nc.tensor.matmul(
                            fp[:, nch * 512:(nch + 1) * 512],
                            lhsT=lhs,
                            rhs=agT[:, k, nch * 512:(nch + 1) * 512],
                            start=(k == 0), stop=(k == KT - 1),
                        )
                ot = outp.tile([128, N], F32, tag="ot", name=f"ot{c}")
                nc.vector.tensor_scalar_add(ot, fp, bo_sb[:, c:c + 1])
                nc.sync.dma_start(out=out_r[:, c, :], in_=ot)

    nc.finalize()
    return nc


def _pad_len(n):
    return max(128, ((n + 127) // 128) * 128)


def _prep_core_inputs(inputs, c, M, idxs):
    b, g = c // 4, c % 4
    rows = slice(g * HD, (g + 1) * HD)
    w_qkv = np.asarray(inputs["w_qkv"], np.float32)
    Wq = (w_qkv[0:H * D][rows]
          + np.asarray(inputs["wq_base"], np.float32)[rows]
          + LS * (np.asarray(inputs["wq_B"], np.float32)[rows]
                  @ np.asarray(inputs["wq_A"], np.float32))) * ATT
    Wk = w_qkv[H * D:2 * H * D][rows]
    Wv = (w_qkv[2 * H * D:3 * H * D][rows]
          + np.asarray(inputs["wv_base"], np.float32)[rows]
          + LS * (np.asarray(inputs["wv_B"], np.float32)[rows]
                  @ np.asarray(inputs["wv_A"], np.float32)))
    wqTv = np.ascontiguousarray(Wq.T).astype(BFNP)
    wkvTv = np.ascontiguousarray(np.concatenate([Wk, Wv], 0).T).astype(BFNP)
    pbqv = (np.asarray(inputs["bq_base"], np.float32)[rows] * ATT).astype(np.float32)
    pbvv = np.asarray(inputs["bv_base"], np.float32)[rows]

    xb = np.asarray(inputs["x"], np.float32)[b]          # [N, DIM]
    xTv = np.ascontiguousarray(xb.T).astype(BFNP)
    idx = idxs[b]
    xk = np.zeros((DIM, M), np.float32)
    xk[:, :len(idx)] = xb[idx].T
    xkTv = xk.astype(BFNP)
    mkv = np.zeros(M, np.float32)
    mkv[:len(idx)] = 1.0

    w_out_slice = np.asarray(inputs["w_out"], np.float32)[rows, :]   # [256, 1024]
    cols = np.concatenate([np.arange(gh * D, (gh + 1) * D)
                           for gh in AG_HEAD_ORDER])
    woTv = np.ascontiguousarray(w_out_slice[:, cols].T).astype(BFNP)
    bov = np.asarray(inputs["b_out"], np.float32)[rows]
    return {"xT": xTv, "xkT": xkTv, "wqT": wqTv, "wkvT": wkvTv,
            "pbq": pbqv, "pbv": pbvv, "mk": mkv, "woT": woTv, "bo": bov}


def kernel(**inputs):
    global LAST_RESULTS
    mask = np.asarray(inputs["mask"]).astype(bool)
    idxs = [np.nonzero(mask[b])[0] for b in range(B)]
    M = _pad_len(max(len(ix) for ix in idxs))
    if M not in _NC_CACHE:
        _NC_CACHE[M] = _build_nc(M)
    nc = _NC_CACHE[M]
    in_maps = [_prep_core_inputs(inputs, c, M, idxs) for c in range(NCORES)]
    res = bass_utils.run_bass_kernel_spmd(
        nc, in_maps, core_ids=list(range(NCORES)),
        trace=TRACE, tmpdir=TRACE_DIR,
    )
    LAST_RESULTS = res
    out = np.empty((B, N, DIM), np.float32)
    for c in range(NCORES):
        b, g = c // 4, c % 4
        out[b, :, g * HD:(g + 1) * HD] = res.results[c]["outT"].T
    return out


# revision 23
# speedup vs baseline: 1.0332x; 1.0332x over previous
"""LoRA attention kernel for 8 Trainium2 NeuronCores.

Sharding: data-parallel over batch B=2 (cores 0-3 -> b=0, cores 4-7 -> b=1),
tensor-parallel over heads within each batch group (4 heads/core). LoRA paths
and q/v base linears are folded host-side into effective projection weights.

Optimizations over the fp32r baseline (668us -> this version):
- All matmul operands bf16 (fp32 PSUM accumulation); tolerance is 2e-2 and
  measured error is ~4e-3, so bf16 is safe and loads/streams/DMAs halve.
- Key/value tokens are compacted host-side using the padding mask (~half of
  the 2048 keys are masked), halving QK/exp/PV work exactly (padded tail
  keys carry mask 0 and contribute zero to both numerator and denominator).
- Scores are computed transposed (ST[m, n]) so the softmax denominator and
  key mask fold into the P@V matmul via an augmented v column.
- Q is kept in a zero-padded [128]-partition layout (each head's 64 dims in
  its parity half, zeros in the other) so QK contracts over 128 partitions:
  K=64 matmuls stream at half rate on the PE, K=128 at full rate.
- One flat software pipeline: the attention stream (QK -> exp -> lagged PV)
  is issued tile-by-tile with the remaining projection work (K/V/Q row tiles
  not needed yet) interleaved in small pieces, so the ACT engine starts
  exp'ing ~40us earlier and the in-order PE queue never blocks on a
  not-ready instruction. PSUM: 2x st[128,1024] + 2x op[65,1024] = 8 banks;
  projection chunks share the st pool's buffers.
- V transposes run on the DMA engine (XBAR dma_start_transpose), not the PE.
- The per-(head,half) drain copies the unnormalized output and denominator
  out of PSUM immediately (freeing the op buffer), then the approximate
  reciprocal (exact one is 6 cyc/elem and stalled the PE 13us/head) and the
  stride-0 DRAM broadcast produce the normalized bf16 attention output.
- Per-head bf16 AllGathers overlap the collectives with later heads'
  compute; gathered-head loads are issued after all attention DMAs so they
  never block the reciprocal round-trips in the FIFO DMA queue.
- The output projection contracts the gathered heads (w_out rows permuted
  host-side to match gather order) right after the last AllGather.
"""

import sys
from contextlib import ExitStack

import numpy as np

for _p in ("/opt/trn_rl_repo", "/opt/trn_rl_repo/concourse"):
    if _p not in sys.path:
        sys.path.insert(0, _p)

import concourse.bass as bass
import concourse.mybir as mybir
import concourse.tile as tile
from concourse import bacc
from concourse import bass_utils

import ml_dtypes

F32 = mybir.dt.float32
BF16 = mybir.dt.bfloat16
EXP = mybir.ActivationFunctionType.Exp
BFNP = ml_dtypes.bfloat16

H, D, DIM, R = 16, 64, 1024, 10
B, N = 2, 2048
NCORES = 8
GROUPS = [[0, 1, 2, 3], [4, 5, 6, 7]]
HPC = H // 4          # heads per core
HD = HPC * D          # 256 projection rows per core
ATT = float(D) ** -0.5
LS = 1.0 / R

KT = DIM // 128       # 8 contraction tiles
NCH = N // 512        # 4 query chunks of 512
# AllGather head order: agout[h] rows are rank-major, so the out-projection
# contraction sees global heads in this order (w_out rows permuted to match).
AG_HEAD_ORDER = [r * HPC + h for h in range(HPC) for r in range(4)]

# test harness hooks
TRACE = False
TRACE_DIR = None
LAST_RESULTS = None

_NC_CACHE = {}


def _chunks(total, step):
    return [(c0, min(c0 + step, total)) for c0 in range(0, total, step)]


def _build_nc(M, debug=False):
    MT = M // 128
    MCH = _chunks(M, 512)
    SEGS = [(h, half) for h in range(HPC) for half in range(2)]
    dbg = "ExternalOutput" if debug else "Internal"

    nc = bacc.Bacc(None, target_bir_lowering=False, num_devices=NCORES)

    xT = nc.dram_tensor("xT", (DIM, N), BF16, kind="ExternalInput")
    xkT = nc.dram_tensor("xkT", (DIM, M), BF16, kind="ExternalInput")
    wqT = nc.dram_tensor("wqT", (DIM, HD), BF16, kind="ExternalInput")
    wkvT = nc.dram_tensor("wkvT", (DIM, 2 * HD), BF16, kind="ExternalInput")
    pbq = nc.dram_tensor("pbq", (HD,), F32, kind="ExternalInput")
    pbv = nc.dram_tensor("pbv", (HD,), F32, kind="ExternalInput")
    mk = nc.dram_tensor("mk", (M,), F32, kind="ExternalInput")
    woT = nc.dram_tensor("woT", (DIM, HD), BF16, kind="ExternalInput")
    bo = nc.dram_tensor("bo", (HD,), F32, kind="ExternalInput")
    outT = nc.dram_tensor("outT", (HD, N), F32, kind="ExternalOutput")

    agin = nc.dram_tensor("agin", (HPC * D, N), BF16)
    agout = nc.dram_tensor("agout", (HPC, 4 * D, N), BF16)
    recd = nc.dram_tensor("recd", (HPC, N), F32, kind=dbg)

    with ExitStack() as ctx:
        tc = ctx.enter_context(tile.TileContext(nc))
        const = ctx.enter_context(tc.tile_pool(name="const", bufs=1))

        pbq_sb = const.tile([128, 2], F32)
        nc.sync.dma_start(out=pbq_sb, in_=pbq[:].rearrange("(i p) -> p i", p=128))
        pbv_sb = const.tile([128, 2], F32)
        nc.sync.dma_start(out=pbv_sb, in_=pbv[:].rearrange("(i p) -> p i", p=128))
        mk_sb = const.tile([128, MT], F32)
        nc.sync.dma_start(out=mk_sb, in_=mk[:].rearrange("(t p) -> p t", p=128))
        bo_sb = const.tile([128, 2], F32)
        nc.sync.dma_start(out=bo_sb, in_=bo[:].rearrange("(c p) -> p c", p=128))
        woT_sb = const.tile([128, KT, HD], BF16)
        woT_r = woT[:, :].rearrange("(k p) c -> p k c", p=128)
        for k in range(KT):
            nc.sync.dma_start(out=woT_sb[:, k, :], in_=woT_r[:, k, :])

        # q in zero-padded per-head layout (see module docstring)
        qz = const.tile([128, HPC, N], BF16)
        nc.vector.memset(qz, 0)
        kvT = const.tile([128, 4, M], BF16)      # [k0 k1 v0 v1] row tiles
        vsb = const.tile([128, MT, HPC, D + 1], BF16)  # v.T + mask column
        agT = const.tile([128, KT, N], BF16)     # gathered heads for out proj

        wq_sb = const.tile([128, KT, HD], BF16)
        wkv_sb = const.tile([128, KT, 2 * HD], BF16)
        xk_sb = const.tile([128, KT, M], BF16)
        xT_sb = const.tile([128, KT, N], BF16)
        wq_r = wqT[:, :].rearrange("(k p) m -> p k m", p=128)
        wkv_r = wkvT[:, :].rearrange("(k p) m -> p k m", p=128)
        xk_r = xkT[:, :].rearrange("(k p) n -> p k n", p=128)
        xT_r = xT[:, :].rearrange("(k p) n -> p k n", p=128)
        # Coarse DMA loads (few big transfers: descriptor issue on the sync
        # engine costs ~0.6us per dma_start, so per-chunk splits are a loss),
        # ordered by consumption: K/V weights + compacted kv tokens first.
        nc.sync.dma_start(out=wkv_sb[:, :, 0:HD], in_=wkv_r[:, :, 0:HD])
        for c0, c1 in MCH:
            nc.sync.dma_start(out=xk_sb[:, :, c0:c1], in_=xk_r[:, :, c0:c1])
        nc.sync.dma_start(out=wkv_sb[:, :, HD:2 * HD], in_=wkv_r[:, :, HD:2 * HD])
        nc.sync.dma_start(out=wq_sb, in_=wq_r)
        for khalf in range(2):
            ksl = slice(khalf * (KT // 2), (khalf + 1) * (KT // 2))
            nc.sync.dma_start(out=xT_sb[:, ksl, :], in_=xT_r[:, ksl, :])

        # ---- phase 1: K/V projections + Q row-tile 0 ----
        # Only K/Q row tile 0 (heads 0-1) must precede the attention stream;
        # row tile 1 (heads 2-3) is interleaved into the stream later.
        with tc.tile_pool(name="vrawp", bufs=4) as vrawp, \
             tc.tile_pool(name="pp_proj", bufs=4, space="PSUM") as ppp:
            pstiles = {}

            def k_chunk(i, ci, pool, klo=0, khi=KT):
                c0, c1 = MCH[ci]
                w = c1 - c0
                key = ("k", i, ci)
                if key not in pstiles:
                    pstiles[key] = pool.tile(
                        [128, 512], F32, tag="ps" if pool is ppp else "st",
                        name=f"psk{i}_{ci}")
                ps = pstiles[key]
                for k in range(klo, khi):
                    nc.tensor.matmul(
                        ps[:, 0:w],
                        lhsT=wkv_sb[:, k, i * 128:(i + 1) * 128],
                        rhs=xk_sb[:, k, c0:c1],
                        start=(k == 0), stop=(k == KT - 1),
                    )
                if khi == KT:
                    nc.vector.tensor_copy(kvT[:, i, c0:c1], ps[:, 0:w])

            def q_chunk(i, nch, pool, klo=0, khi=KT):
                sl = slice(nch * 512, (nch + 1) * 512)
                key = ("q", i, nch)
                if key not in pstiles:
                    pstiles[key] = pool.tile(
                        [128, 512], F32, tag="ps" if pool is ppp else "st",
                        name=f"psq{i}_{nch}")
                ps = pstiles[key]
                for k in range(klo, khi):
                    nc.tensor.matmul(
                        ps,
                        lhsT=wq_sb[:, k, i * 128:(i + 1) * 128],
                        rhs=xT_sb[:, k, sl],
                        start=(k == 0), stop=(k == KT - 1),
                    )
                if khi == KT:
                    nc.vector.tensor_scalar_add(
                        qz[0:64, 2 * i, sl], ps[0:64, :], pbq_sb[0:64, i:i + 1])
                    nc.vector.tensor_scalar_add(
                        qz[64:128, 2 * i + 1, sl], ps[64:128, :],
                        pbq_sb[64:128, i:i + 1])

            for ci in range(len(MCH)):
                k_chunk(0, ci, ppp)
            # V projection (+bias)
            for i in range(2):
                for ci, (c0, c1) in enumerate(MCH):
                    w = c1 - c0
                    ps = ppp.tile([128, 512], F32, tag="ps", name=f"psv{i}_{ci}")
                    for k in range(KT):
                        nc.tensor.matmul(
                            ps[:, 0:w],
                            lhsT=wkv_sb[:, k, HD + i * 128:HD + (i + 1) * 128],
                            rhs=xk_sb[:, k, c0:c1],
                            start=(k == 0), stop=(k == KT - 1),
                        )
                    nc.vector.tensor_scalar_add(
                        kvT[:, 2 + i, c0:c1], ps[:, 0:w], pbv_sb[:, i:i + 1])
            for nch in range(NCH):
                q_chunk(0, nch, ppp)
            # V transpose on the DMA engine (XBAR), j-major so the j=0 pass
            # only waits on V row-tile 0; mask-scale rows and set the
            # augmented mask column.
            for j in range(2):
                for t in range(MT):
                    vr = vrawp.tile([128, 128], BF16, tag="vraw",
                                    name=f"vr{t}_{j}")
                    nc.sync.dma_start_transpose(
                        out=vr, in_=kvT[:, 2 + j, t * 128:(t + 1) * 128])
                    for hh in range(2):
                        h = j * 2 + hh
                        nc.vector.tensor_scalar_mul(
                            vsb[:, t, h, 0:D],
                            vr[:, hh * D:(hh + 1) * D],
                            mk_sb[:, t:t + 1],
                        )
                    if j == 1:
                        for h in range(HPC):
                            nc.vector.tensor_copy(vsb[:, t, h, D:D + 1],
                                                  mk_sb[:, t:t + 1])

        # ---- phase 2: attention stream (flat pipeline, PV lags QK by 2) ----
        with tc.tile_pool(name="expool", bufs=6) as expool, \
             tc.tile_pool(name="attup", bufs=2) as attup, \
             tc.tile_pool(name="attp", bufs=2) as attp, \
             tc.tile_pool(name="recbp", bufs=2) as recbp, \
             tc.tile_pool(name="recp", bufs=1) as recp, \
             tc.tile_pool(name="pp_o", bufs=2, space="PSUM") as ppo, \
             tc.tile_pool(name="pp_st", bufs=2, space="PSUM") as ppst:

            def drain(op, h, half, nsl):
                # Copy the unnormalized output + denominator out of PSUM
                # right away (frees the op buffer), then normalize via the
                # approximate reciprocal and a stride-0 DRAM broadcast.
                att_u = attup.tile([D, 1024], BF16, tag="attu",
                                   name=f"attu{h}_{half}")
                den = recp.tile([1, 1024], F32, tag="den",
                                name=f"den{h}_{half}")
                nc.vector.tensor_copy(den, op[D:D + 1, :])
                nc.vector.tensor_copy(att_u, op[0:D, :])
                rec = recp.tile([1, 1024], F32, tag="rec",
                                name=f"rec{h}_{half}")
                nc.vector.reciprocal_approx_fast(rec, den)
                nc.sync.dma_start(out=recd[h:h + 1, nsl], in_=rec)
                recb = recbp.tile([D, 1024], F32, tag="recb",
                                  name=f"recb{h}_{half}")
                rsrc = recd[h:h + 1, nsl]
                nc.sync.dma_start(
                    out=recb,
                    in_=bass.AP(tensor=rsrc.tensor, offset=rsrc.offset,
                                ap=[[0, D], [1, 1024]]),
                )
                att = attp.tile([D, 1024], BF16, tag="att",
                                name=f"att{h}_{half}")
                nc.vector.tensor_mul(att, att_u, recb)
                nc.sync.dma_start(out=agin[h * D:(h + 1) * D, nsl], in_=att)
                if half == 1:
                    nc.gpsimd.collective_compute(
                        "AllGather", mybir.AluOpType.bypass,
                        replica_groups=GROUPS,
                        ins=[agin[h * D:(h + 1) * D, :].opt()],
                        outs=[agout[h, :, :].opt()],
                    )

            pend = []

            def make_pv(op, h, half, t, ex, nsl):
                def f():
                    for c in range(2):
                        nc.tensor.matmul(
                            op[:, c * 512:(c + 1) * 512],
                            lhsT=vsb[:, t, h, :],
                            rhs=ex[:, c * 512:(c + 1) * 512],
                            start=(t == 0), stop=(t == MT - 1),
                        )
                    if t == MT - 1:
                        drain(op, h, half, nsl)
                return f

            projq = []
            for ci in range(len(MCH)):
                projq += [("k", 1, ci, 0, 4), ("k", 1, ci, 4, KT)]
            for nch in range(NCH):
                projq += [("q", 1, nch, 0, 4), ("q", 1, nch, 4, KT)]

            for si, (h, half) in enumerate(SEGS):
                nsl = slice(half * 1024, (half + 1) * 1024)
                op = ppo.tile([D + 1, 1024], F32, tag="op",
                              name=f"op{h}_{half}")
                for t in range(MT):
                    st = ppst.tile([128, 1024], F32, tag="st",
                                   name=f"st{h}_{half}_{t}")
                    for c in range(2):
                        nc.tensor.matmul(
                            st[:, c * 512:(c + 1) * 512],
                            lhsT=kvT[:, h // 2, t * 128:(t + 1) * 128],
                            rhs=qz[:, h, half * 1024 + c * 512:
                                   half * 1024 + (c + 1) * 512],
                            start=True, stop=True,
                        )
                    ex = expool.tile([128, 1024], BF16, tag="ex",
                                     name=f"ex{h}_{half}_{t}")
                    nc.scalar.activation(ex, st, EXP)
                    if si >= 2 and projq:
                        p = projq.pop(0)
                        if p[0] == "k":
                            k_chunk(p[1], p[2], ppst, p[3], p[4])
                        else:
                            q_chunk(p[1], p[2], ppst, p[3], p[4])
                    pend.append(make_pv(op, h, half, t, ex, nsl))
                    if len(pend) > 2:
                        pend.pop(0)()
            while pend:
                pend.pop(0)()

        # ---- output projection slice ----
        with tc.tile_pool(name="outp", bufs=2) as outp, \
             tc.tile_pool(name="pp_f", bufs=2, space="PSUM") as ppf:
            # gathered-head loads issued after all attention DMAs: the DMA
            # queue is FIFO, so issuing them earlier would block the
            # reciprocal round-trips behind collective waits.
            for h in range(HPC):
                ag_r = agout[h, :, :].rearrange("(kk p) n -> p kk n", p=128)
                for kk in range(2):
                    nc.sync.dma_start(out=agT[:, 2 * h + kk, :],
                                      in_=ag_r[:, kk, :])
            out_r = outT[:, :].rearrange("(c p) n -> p c n", p=128)
            for c in range(2):
                fp = ppf.tile([128, N], F32, tag="fp", name=f"fp{c}")
                for k in range(KT):
                    lhs = woT_sb[:, k, c * 128:(c + 1) * 128]
                    for nch in range(NCH):
                        nc.tensor.matmul(
                            fp[:, nch * 512:(nch + 1) * 512],
                            lhsT=lhs,
                            rhs=agT[:, k, nch * 512:(nch + 1) * 512],
                            start=(k == 0), stop=(k == KT - 1),
                        )
                ot = outp.tile([128, N], F32, tag="ot", name=f"ot{c}")
                nc.vector.tensor_scalar_add(ot, fp, bo_sb[:, c:c + 1])
                nc.sync.dma_start(out=out_r[:, c, :], in_=ot)

    nc.finalize()
    return nc


def _pad_len(n):
    return max(128, ((n + 127) // 128) * 128)


def _prep_core_inputs(inputs, c, M, idxs):
    b, g = c // 4, c % 4
    rows = slice(g * HD, (g + 1) * HD)
    w_qkv = np.asarray(inputs["w_qkv"], np.float32)
    Wq = (w_qkv[0:H * D][rows]
          + np.asarray(inputs["wq_base"], np.float32)[rows]
          + LS * (np.asarray(inputs["wq_B"], np.float32)[rows]
                  @ np.asarray(inputs["wq_A"], np.float32))) * ATT
    Wk = w_qkv[H * D:2 * H * D][rows]
    Wv = (w_qkv[2 * H * D:3 * H * D][rows]
          + np.asarray(inputs["wv_base"], np.float32)[rows]
          + LS * (np.asarray(inputs["wv_B"], np.float32)[rows]
                  @ np.asarray(inputs["wv_A"], np.float32)))
    wqTv = np.ascontiguousarray(Wq.T).astype(BFNP)
    wkvTv = np.ascontiguousarray(np.concatenate([Wk, Wv], 0).T).astype(BFNP)
    pbqv = (np.asarray(inputs["bq_base"], np.float32)[rows] * ATT).astype(np.float32)
    pbvv = np.asarray(inputs["bv_base"], np.float32)[rows]

    xb = np.asarray(inputs["x"], np.float32)[b]          # [N, DIM]
    xTv = np.ascontiguousarray(xb.T).astype(BFNP)
    idx = idxs[b]
    xk = np.zeros((DIM, M), np.float32)
    xk[:, :len(idx)] = xb[idx].T
    xkTv = xk.astype(BFNP)
    mkv = np.zeros(M, np.float32)
    mkv[:len(idx)] = 1.0

    w_out_slice = np.asarray(inputs["w_out"], np.float32)[rows, :]   # [256, 1024]
    cols = np.concatenate([np.arange(gh * D, (gh + 1) * D)
                           for gh in AG_HEAD_ORDER])
    woTv = np.ascontiguousarray(w_out_slice[:, cols].T).astype(BFNP)
    bov = np.asarray(inputs["b_out"], np.float32)[rows]
    return {"xT": xTv, "xkT": xkTv, "wqT": wqTv, "wkvT": wkvTv,
            "pbq": pbqv, "pbv": pbvv, "mk": mkv, "woT": woTv, "bo": bov}


def kernel(**inputs):
    global LAST_RESULTS
    mask = np.asarray(inputs["mask"]).astype(bool)
    idxs = [np.nonzero(mask[b])[0] for b in range(B)]
    M = _pad_len(max(len(ix) for ix in idxs))
    if M not in _NC_CACHE:
        _NC_CACHE[M] = _build_nc(M)
    nc = _NC_CACHE[M]
    in_maps = [_prep_core_inputs(inputs, c, M, idxs) for c in range(NCORES)]
    res = bass_utils.run_bass_kernel_spmd(
        nc, in_maps, core_ids=list(range(NCORES)),
        trace=TRACE, tmpdir=TRACE_DIR,
    )
    LAST_RESULTS = res
    out = np.empty((B, N, DIM), np.float32)
    for c in range(NCORES):
        b, g = c // 4, c % 4
        out[b, :, g * HD:(g + 1) * HD] = res.results[c]["outT"].T
    return out


# revision 24
# speedup vs baseline: 1.0922x; 1.0571x over previous
"""LoRA attention kernel for 8 Trainium2 NeuronCores.

Sharding: data-parallel over batch B=2 (cores 0-3 -> b=0, cores 4-7 -> b=1),
tensor-parallel over heads within each batch group (4 heads/core). LoRA paths
and q/v base linears are folded host-side into effective projection weights.

Optimizations over the fp32r baseline (668us -> this version):
- All matmul operands bf16 (fp32 PSUM accumulation); tolerance is 2e-2 and
  measured error is ~4e-3, so bf16 is safe and loads/streams/DMAs halve.
- Key/value tokens are compacted host-side using the padding mask (~half of
  the 2048 keys are masked), halving QK/exp/PV work exactly (padded tail
  keys carry mask 0 and contribute zero to both numerator and denominator).
- Scores are computed transposed (ST[m, n]) so the softmax denominator and
  key mask fold into the P@V matmul via an augmented v column.
- Q is kept in a zero-padded [128]-partition layout (each head's 64 dims in
  its parity half, zeros in the other) so QK contracts over 128 partitions:
  K=64 matmuls stream at half rate on the PE, K=128 at full rate.
- One flat software pipeline: the attention stream (QK -> exp -> lagged PV)
  is issued tile-by-tile with the remaining projection work (K/V/Q row tiles
  not needed yet) interleaved in small pieces, so the ACT engine starts
  exp'ing ~40us earlier and the in-order PE queue never blocks on a
  not-ready instruction. PSUM: 2x st[128,1024] + 2x op[65,1024] = 8 banks;
  projection chunks share the st pool's buffers.
- V transposes run on the DMA engine (XBAR dma_start_transpose), not the PE.
- The per-(head,half) drain copies the unnormalized output and denominator
  out of PSUM immediately (freeing the op buffer), then the approximate
  reciprocal (exact one is 6 cyc/elem and stalled the PE 13us/head) and the
  stride-0 DRAM broadcast produce the normalized bf16 attention output.
- Per-head bf16 AllGathers overlap the collectives with later heads'
  compute; gathered-head loads are issued after all attention DMAs so they
  never block the reciprocal round-trips in the FIFO DMA queue.
- The output projection contracts the gathered heads (w_out rows permuted
  host-side to match gather order) right after the last AllGather.
"""

import sys
from contextlib import ExitStack

import numpy as np

for _p in ("/opt/trn_rl_repo", "/opt/trn_rl_repo/concourse"):
    if _p not in sys.path:
        sys.path.insert(0, _p)

import concourse.bass as bass
import concourse.mybir as mybir
import concourse.tile as tile
from concourse import bacc
from concourse import bass_utils

import ml_dtypes

F32 = mybir.dt.float32
BF16 = mybir.dt.bfloat16
EXP = mybir.ActivationFunctionType.Exp
BFNP = ml_dtypes.bfloat16

H, D, DIM, R = 16, 64, 1024, 10
B, N = 2, 2048
NCORES = 8
GROUPS = [[0, 1, 2, 3], [4, 5, 6, 7]]
HPC = H // 4          # heads per core
HD = HPC * D          # 256 projection rows per core
ATT = float(D) ** -0.5
LS = 1.0 / R

KT = DIM // 128       # 8 contraction tiles
NCH = N // 512        # 4 query chunks of 512
# AllGather head order: agout[h] rows are rank-major, so the out-projection
# contraction sees global heads in this order (w_out rows permuted to match).
AG_HEAD_ORDER = [r * HPC + h for h in range(HPC) for r in range(4)]

# test harness hooks
TRACE = False
TRACE_DIR = None
LAST_RESULTS = None

_NC_CACHE = {}


def _chunks(total, step):
    return [(c0, min(c0 + step, total)) for c0 in range(0, total, step)]


def _build_nc(M, debug=False):
    MT = M // 128
    MCH = _chunks(M, 512)
    SEGS = [(h, half) for h in range(HPC) for half in range(2)]
    dbg = "ExternalOutput" if debug else "Internal"

    nc = bacc.Bacc(None, target_bir_lowering=False, num_devices=NCORES)

    xT = nc.dram_tensor("xT", (DIM, N), BF16, kind="ExternalInput")
    xkT = nc.dram_tensor("xkT", (DIM, M), BF16, kind="ExternalInput")
    wqT = nc.dram_tensor("wqT", (DIM, HD), BF16, kind="ExternalInput")
    wkvT = nc.dram_tensor("wkvT", (DIM, 2 * HD), BF16, kind="ExternalInput")
    pbq = nc.dram_tensor("pbq", (HD,), F32, kind="ExternalInput")
    pbv = nc.dram_tensor("pbv", (HD,), F32, kind="ExternalInput")
    mk = nc.dram_tensor("mk", (M,), F32, kind="ExternalInput")
    woT = nc.dram_tensor("woT", (DIM, HD), BF16, kind="ExternalInput")
    bo = nc.dram_tensor("bo", (HD,), F32, kind="ExternalInput")
    outT = nc.dram_tensor("outT", (HD, N), F32, kind="ExternalOutput")

    agin = nc.dram_tensor("agin", (HPC * D, N), BF16)
    agout = nc.dram_tensor("agout", (HPC, 4 * D, N), BF16)
    recd = nc.dram_tensor("recd", (HPC, N), F32, kind=dbg)

    with ExitStack() as ctx:
        tc = ctx.enter_context(tile.TileContext(nc))
        const = ctx.enter_context(tc.tile_pool(name="const", bufs=1))

        pbq_sb = const.tile([128, 2], F32)
        nc.sync.dma_start(out=pbq_sb, in_=pbq[:].rearrange("(i p) -> p i", p=128))
        pbv_sb = const.tile([128, 2], F32)
        nc.sync.dma_start(out=pbv_sb, in_=pbv[:].rearrange("(i p) -> p i", p=128))
        mk_sb = const.tile([128, MT], F32)
        nc.sync.dma_start(out=mk_sb, in_=mk[:].rearrange("(t p) -> p t", p=128))
        bo_sb = const.tile([128, 2], F32)
        nc.sync.dma_start(out=bo_sb, in_=bo[:].rearrange("(c p) -> p c", p=128))
        woT_sb = const.tile([128, KT, HD], BF16)
        woT_r = woT[:, :].rearrange("(k p) c -> p k c", p=128)
        for k in range(KT):
            nc.sync.dma_start(out=woT_sb[:, k, :], in_=woT_r[:, k, :])

        # q in zero-padded per-head layout (see module docstring)
        qz = const.tile([128, HPC, N], BF16)
        nc.vector.memset(qz, 0)
        kvT = const.tile([128, 4, M], BF16)      # [k0 k1 v0 v1] row tiles
        vsb = const.tile([128, MT, HPC, D + 1], BF16)  # v.T + mask column
        agT = const.tile([128, KT, N], BF16)     # gathered heads for out proj

        wq_sb = const.tile([128, KT, HD], BF16)
        wkv_sb = const.tile([128, KT, 2 * HD], BF16)
        xk_sb = const.tile([128, KT, M], BF16)
        xT_sb = const.tile([128, KT, N], BF16)
        wq_r = wqT[:, :].rearrange("(k p) m -> p k m", p=128)
        wkv_r = wkvT[:, :].rearrange("(k p) m -> p k m", p=128)
        xk_r = xkT[:, :].rearrange("(k p) n -> p k n", p=128)
        xT_r = xT[:, :].rearrange("(k p) n -> p k n", p=128)
        # Coarse DMA loads (few big transfers: descriptor issue on the sync
        # engine costs ~0.6us per dma_start, so per-chunk splits are a loss),
        # ordered by consumption: K/V weights + compacted kv tokens first.
        nc.sync.dma_start(out=wkv_sb[:, :, 0:HD], in_=wkv_r[:, :, 0:HD])
        for c0, c1 in MCH:
            nc.sync.dma_start(out=xk_sb[:, :, c0:c1], in_=xk_r[:, :, c0:c1])
        nc.sync.dma_start(out=wkv_sb[:, :, HD:2 * HD], in_=wkv_r[:, :, HD:2 * HD])
        nc.sync.dma_start(out=wq_sb, in_=wq_r)
        for khalf in range(2):
            ksl = slice(khalf * (KT // 2), (khalf + 1) * (KT // 2))
            nc.sync.dma_start(out=xT_sb[:, ksl, :], in_=xT_r[:, ksl, :])

        # vrawp stays open for the whole kernel: closing it before the
        # attention stream would make the stream pools' first tiles (which
        # reuse its bytes) wait on the mask-muls that trail the serial XBAR
        # transposes, stalling the PE ~19us before the first QK.
        vrawp = ctx.enter_context(tc.tile_pool(name="vrawp", bufs=4))

        # ---- phase 1: K/V projections + Q row-tile 0 ----
        # Only K/Q row tile 0 (heads 0-1) must precede the attention stream;
        # row tile 1 (heads 2-3) is interleaved into the stream later.
        with tc.tile_pool(name="pp_proj", bufs=4, space="PSUM") as ppp:
            pstiles = {}

            def k_chunk(i, ci, pool, klo=0, khi=KT):
                c0, c1 = MCH[ci]
                w = c1 - c0
                key = ("k", i, ci)
                if key not in pstiles:
                    pstiles[key] = pool.tile(
                        [128, 512], F32, tag="ps" if pool is ppp else "st",
                        name=f"psk{i}_{ci}")
                ps = pstiles[key]
                for k in range(klo, khi):
                    nc.tensor.matmul(
                        ps[:, 0:w],
                        lhsT=wkv_sb[:, k, i * 128:(i + 1) * 128],
                        rhs=xk_sb[:, k, c0:c1],
                        start=(k == 0), stop=(k == KT - 1),
                    )
                if khi == KT:
                    nc.vector.tensor_copy(kvT[:, i, c0:c1], ps[:, 0:w])

            def q_chunk(i, nch, pool, klo=0, khi=KT):
                sl = slice(nch * 512, (nch + 1) * 512)
                key = ("q", i, nch)
                if key not in pstiles:
                    pstiles[key] = pool.tile(
                        [128, 512], F32, tag="ps" if pool is ppp else "st",
                        name=f"psq{i}_{nch}")
                ps = pstiles[key]
                for k in range(klo, khi):
                    nc.tensor.matmul(
                        ps,
                        lhsT=wq_sb[:, k, i * 128:(i + 1) * 128],
                        rhs=xT_sb[:, k, sl],
                        start=(k == 0), stop=(k == KT - 1),
                    )
                if khi == KT:
                    nc.vector.tensor_scalar_add(
                        qz[0:64, 2 * i, sl], ps[0:64, :], pbq_sb[0:64, i:i + 1])
                    nc.vector.tensor_scalar_add(
                        qz[64:128, 2 * i + 1, sl], ps[64:128, :],
                        pbq_sb[64:128, i:i + 1])

            for ci in range(len(MCH)):
                k_chunk(0, ci, ppp)
            # V projection (+bias)
            for i in range(2):
                for ci, (c0, c1) in enumerate(MCH):
                    w = c1 - c0
                    ps = ppp.tile([128, 512], F32, tag="ps", name=f"psv{i}_{ci}")
                    for k in range(KT):
                        nc.tensor.matmul(
                            ps[:, 0:w],
                            lhsT=wkv_sb[:, k, HD + i * 128:HD + (i + 1) * 128],
                            rhs=xk_sb[:, k, c0:c1],
                            start=(k == 0), stop=(k == KT - 1),
                        )
                    nc.vector.tensor_scalar_add(
                        kvT[:, 2 + i, c0:c1], ps[:, 0:w], pbv_sb[:, i:i + 1])
            for nch in range(NCH):
                q_chunk(0, nch, ppp)
            # V transpose on the DMA engine (XBAR), j-major so the j=0 pass
            # only waits on V row-tile 0; mask-scale rows and set the
            # augmented mask column.
            for j in range(2):
                for t in range(MT):
                    vr = vrawp.tile([128, 128], BF16, tag="vraw",
                                    name=f"vr{t}_{j}")
                    nc.sync.dma_start_transpose(
                        out=vr, in_=kvT[:, 2 + j, t * 128:(t + 1) * 128])
                    for hh in range(2):
                        h = j * 2 + hh
                        nc.vector.tensor_scalar_mul(
                            vsb[:, t, h, 0:D],
                            vr[:, hh * D:(hh + 1) * D],
                            mk_sb[:, t:t + 1],
                        )
                    if j == 1:
                        for h in range(HPC):
                            nc.vector.tensor_copy(vsb[:, t, h, D:D + 1],
                                                  mk_sb[:, t:t + 1])

        # ---- phase 2: attention stream (flat pipeline, PV lags QK by 2) ----
        with tc.tile_pool(name="expool", bufs=6) as expool, \
             tc.tile_pool(name="attup", bufs=2) as attup, \
             tc.tile_pool(name="attp", bufs=2) as attp, \
             tc.tile_pool(name="recbp", bufs=2) as recbp, \
             tc.tile_pool(name="recp", bufs=1) as recp, \
             tc.tile_pool(name="pp_o", bufs=2, space="PSUM") as ppo, \
             tc.tile_pool(name="pp_st", bufs=2, space="PSUM") as ppst:

            def drain(op, h, half, nsl):
                # Copy the unnormalized output + denominator out of PSUM
                # right away (frees the op buffer), then normalize via the
                # approximate reciprocal and a stride-0 DRAM broadcast.
                att_u = attup.tile([D, 1024], BF16, tag="attu",
                                   name=f"attu{h}_{half}")
                den = recp.tile([1, 1024], F32, tag="den",
                                name=f"den{h}_{half}")
                nc.vector.tensor_copy(den, op[D:D + 1, :])
                nc.vector.tensor_copy(att_u, op[0:D, :])
                rec = recp.tile([1, 1024], F32, tag="rec",
                                name=f"rec{h}_{half}")
                nc.vector.reciprocal_approx_fast(rec, den)
                nc.sync.dma_start(out=recd[h:h + 1, nsl], in_=rec)
                recb = recbp.tile([D, 1024], F32, tag="recb",
                                  name=f"recb{h}_{half}")
                rsrc = recd[h:h + 1, nsl]
                nc.sync.dma_start(
                    out=recb,
                    in_=bass.AP(tensor=rsrc.tensor, offset=rsrc.offset,
                                ap=[[0, D], [1, 1024]]),
                )
                att = attp.tile([D, 1024], BF16, tag="att",
                                name=f"att{h}_{half}")
                nc.vector.tensor_mul(att, att_u, recb)
                nc.sync.dma_start(out=agin[h * D:(h + 1) * D, nsl], in_=att)
                if half == 1:
                    nc.gpsimd.collective_compute(
                        "AllGather", mybir.AluOpType.bypass,
                        replica_groups=GROUPS,
                        ins=[agin[h * D:(h + 1) * D, :].opt()],
                        outs=[agout[h, :, :].opt()],
                    )

            pend = []

            def make_pv(op, h, half, t, ex, nsl):
                def f():
                    for c in range(2):
                        nc.tensor.matmul(
                            op[:, c * 512:(c + 1) * 512],
                            lhsT=vsb[:, t, h, :],
                            rhs=ex[:, c * 512:(c + 1) * 512],
                            start=(t == 0), stop=(t == MT - 1),
                        )
                    if t == MT - 1:
                        drain(op, h, half, nsl)
                return f

            projq = []
            for ci in range(len(MCH)):
                projq += [("k", 1, ci, 0, 4), ("k", 1, ci, 4, KT)]
            for nch in range(NCH):
                projq += [("q", 1, nch, 0, 4), ("q", 1, nch, 4, KT)]

            for si, (h, half) in enumerate(SEGS):
                nsl = slice(half * 1024, (half + 1) * 1024)
                op = ppo.tile([D + 1, 1024], F32, tag="op",
                              name=f"op{h}_{half}")
                for t in range(MT):
                    st = ppst.tile([128, 1024], F32, tag="st",
                                   name=f"st{h}_{half}_{t}")
                    for c in range(2):
                        nc.tensor.matmul(
                            st[:, c * 512:(c + 1) * 512],
                            lhsT=kvT[:, h // 2, t * 128:(t + 1) * 128],
                            rhs=qz[:, h, half * 1024 + c * 512:
                                   half * 1024 + (c + 1) * 512],
                            start=True, stop=True,
                        )
                    ex = expool.tile([128, 1024], BF16, tag="ex",
                                     name=f"ex{h}_{half}_{t}")
                    nc.scalar.activation(ex, st, EXP)
                    if si >= 2 and projq:
                        p = projq.pop(0)
                        if p[0] == "k":
                            k_chunk(p[1], p[2], ppst, p[3], p[4])
                        else:
                            q_chunk(p[1], p[2], ppst, p[3], p[4])
                    pend.append(make_pv(op, h, half, t, ex, nsl))
                    if len(pend) > 2:
                        pend.pop(0)()
            while pend:
                pend.pop(0)()

        # ---- output projection slice ----
        with tc.tile_pool(name="outp", bufs=2) as outp, \
             tc.tile_pool(name="pp_f", bufs=2, space="PSUM") as ppf:
            # gathered-head loads issued after all attention DMAs: the DMA
            # queue is FIFO, so issuing them earlier would block the
            # reciprocal round-trips behind collective waits.
            for h in range(HPC):
                ag_r = agout[h, :, :].rearrange("(kk p) n -> p kk n", p=128)
                for kk in range(2):
                    nc.sync.dma_start(out=agT[:, 2 * h + kk, :],
                                      in_=ag_r[:, kk, :])
            out_r = outT[:, :].rearrange("(c p) n -> p c n", p=128)
            for c in range(2):
                fp = ppf.tile([128, N], F32, tag="fp", name=f"fp{c}")
                for k in range(KT):
                    lhs = woT_sb[:, k, c * 128:(c + 1) * 128]
                    for nch in range(NCH):
                        nc.tensor.matmul(
                            fp[:, nch * 512:(nch + 1) * 512],
                            lhsT=lhs,
                            rhs=agT[:, k, nch * 512:(nch + 1) * 512],
                            start=(k == 0), stop=(k == KT - 1),
                        )
                ot = outp.tile([128, N], F32, tag="ot", name=f"ot{c}")
                nc.vector.tensor_scalar_add(ot, fp, bo_sb[:, c:c + 1])
                nc.sync.dma_start(out=out_r[:, c, :], in_=ot)

    nc.finalize()
    return nc


def _pad_len(n):
    return max(128, ((n + 127) // 128) * 128)


def _prep_core_inputs(inputs, c, M, idxs):
    b, g = c // 4, c % 4
    rows = slice(g * HD, (g + 1) * HD)
    w_qkv = np.asarray(inputs["w_qkv"], np.float32)
    Wq = (w_qkv[0:H * D][rows]
          + np.asarray(inputs["wq_base"], np.float32)[rows]
          + LS * (np.asarray(inputs["wq_B"], np.float32)[rows]
                  @ np.asarray(inputs["wq_A"], np.float32))) * ATT
    Wk = w_qkv[H * D:2 * H * D][rows]
    Wv = (w_qkv[2 * H * D:3 * H * D][rows]
          + np.asarray(inputs["wv_base"], np.float32)[rows]
          + LS * (np.asarray(inputs["wv_B"], np.float32)[rows]
                  @ np.asarray(inputs["wv_A"], np.float32)))
    wqTv = np.ascontiguousarray(Wq.T).astype(BFNP)
    wkvTv = np.ascontiguousarray(np.concatenate([Wk, Wv], 0).T).astype(BFNP)
    pbqv = (np.asarray(inputs["bq_base"], np.float32)[rows] * ATT).astype(np.float32)
    pbvv = np.asarray(inputs["bv_base"], np.float32)[rows]

    xb = np.asarray(inputs["x"], np.float32)[b]          # [N, DIM]
    xTv = np.ascontiguousarray(xb.T).astype(BFNP)
    idx = idxs[b]
    xk = np.zeros((DIM, M), np.float32)
    xk[:, :len(idx)] = xb[idx].T
    xkTv = xk.astype(BFNP)
    mkv = np.zeros(M, np.float32)
    mkv[:len(idx)] = 1.0

    w_out_slice = np.asarray(inputs["w_out"], np.float32)[rows, :]   # [256, 1024]
    cols = np.concatenate([np.arange(gh * D, (gh + 1) * D)
                           for gh in AG_HEAD_ORDER])
    woTv = np.ascontiguousarray(w_out_slice[:, cols].T).astype(BFNP)
    bov = np.asarray(inputs["b_out"], np.float32)[rows]
    return {"xT": xTv, "xkT": xkTv, "wqT": wqTv, "wkvT": wkvTv,
            "pbq": pbqv, "pbv": pbvv, "mk": mkv, "woT": woTv, "bo": bov}


def kernel(**inputs):
    global LAST_RESULTS
    mask = np.asarray(inputs["mask"]).astype(bool)
    idxs = [np.nonzero(mask[b])[0] for b in range(B)]
    M = _pad_len(max(len(ix) for ix in idxs))
    if M not in _NC_CACHE:
        _NC_CACHE[M] = _build_nc(M)
    nc = _NC_CACHE[M]
    in_maps = [_prep_core_inputs(inputs, c, M, idxs) for c in range(NCORES)]
    res = bass_utils.run_bass_kernel_spmd(
        nc, in_maps, core_ids=list(range(NCORES)),
        trace=TRACE, tmpdir=TRACE_DIR,
    )
    LAST_RESULTS = res
    out = np.empty((B, N, DIM), np.float32)
    for c in range(NCORES):
        b, g = c // 4, c % 4
        out[b, :, g * HD:(g + 1) * HD] = res.results[c]["outT"].T
    return out
